# revision 65
# baseline (speedup 1.0000x reference)
"""Window attention (BaseWindowAttention) Trainium2 kernel, v2.

Data-parallel over the 8 (b,l) slices, one NeuronCore each. Host prep:
transpose each slice to [c, tok] with tokens in window order, quantize to
fp8e4 (x split hi+lo at scale 16, weights at scale 64) packed for DoubleRow
matmuls: channel c = 256*kt2 + 128*i + p lives at [partition p, pair slot i]
of k-tile-pair kt2. Cost notes: PE matmul time = out-free-size x cycles/row
(bf16 1.0, fp8 DoubleRow 0.5, independent of K), so fp8 DR quarters the
qk-projection PE time and rank-64 bias accumulate rides free in K.

Device pipeline per chunk (1024 tokens = 2 octs of 8 windows):
  stage 1: qk projection = 1-pass fp8 DoubleRow (2 matmuls per f-tile-half,
           ~1.1% extra rel err, total 1.24e-2 vs 2e-2 gate); v projection =
           3-pass hi/lo-compensated fp8 DoubleRow (exact to ~bf16); ones
           column memset for the softmax denominators. PSUM->SBUF copies
           balance ACT (most) vs DVE (K_QKDVE f-tiles at th1).
  dots:    per even/odd head pair, the relative-position bias enters PSUM
           first via one rank-64 SVD matmul per 64-row half (factors
           pre-scaled by 2^10 each to match the fp8 scale product 2^20),
           dots accumulate on top (no DVE bias adds); exp on ACT with
           scale SCALE/2^20.
  mm2:     ones-column appended to v puts the denominators in PSUM row 64;
           DVE reciprocal -> gpsimd partition_broadcast (on-chip, no DRAM
           round trip); DVE tensor_tensor mult normalizes into the at
           tiles (odd head via an SP-queue SBUF DMA for the partition
           shift).
  oproj:   bf16 matmuls; PSUM->SBUF copies split ACT/DVE, out DMAs on
           SP/Pool.

Drain (last chunk): odd-head norms for m7 (and optionally m5-oct1) skip
their shift DMA - the oproj reads those alo tiles directly via wout2, a
host-side duplicate of w_out's odd-head row halves at partitions 0-63,
with the kt3 matmul split into two K=64 halves. Even-head norms of the
final oct run as ACT copy + Pool multiply (DVE's recip chain paces the
drain). kt0-2 are prestaged per-kt; the final copies/out-DMAs split
across ACT+DVE / SP+Pool+ACT queues.

Startup: PE-ramp warmup matmul + ACT Exp-table warm (their memsets must
precede the DMA emissions or the in-order ACT queue stalls); bias factors
ride the idle ACT queue; the slow wout/wout2 DMA setups (~1.6us each) are
deferred to pipeline iteration 2 on SP/Pool - on the ACT queue at t=0
they delay the first qk copies by ~2.6us.

Engine budget (cost model, per core): ACT ~110us (exp + copies) is the
pacer at 92% busy; PE ~105us; DVE ~96us; SP/Pool ~50us.

Backend landmines (bisected): column tile_position crashes the device;
mixing tile_position rows within one PSUM tile crashes the device; AluOp
divide does not compile; tensor_tensor cannot read two PSUM operands;
GPSIMD cannot access PSUM (BIR verifier); DMA cannot read PSUM;
partition-stride-0 APs are rejected outside DMA/partition_broadcast.

Self-contained: shapes hardcoded, no sibling imports.
"""
import os
import numpy as np
import ml_dtypes

import concourse.mybir as mybir
import concourse.tile as tile
from concourse import bacc
from concourse.bass_utils import run_bass_kernel_spmd

F32 = mybir.dt.float32
BF16 = mybir.dt.bfloat16
F8 = mybir.dt.float8e4
NPF8 = ml_dtypes.float8_e4m3

B, L, H, W, C = 2, 4, 64, 64, 512
HEADS, CH, WS = 8, 64, 8
WTOK = WS * WS                        # 64 tokens per window
TOK = H * W                           # 4096 tokens per slice
INNER = HEADS * CH                    # 512
SCALE = CH ** -0.5                    # 0.125
CHUNK = 1024                          # tokens per pipeline chunk (16 windows)
NCHUNK = TOK // CHUNK                 # 4
NUNITS = 16                           # attention units per chunk (8 heads x 2)
NCORES = 8
SX, SW = 16.0, 64.0                   # fp8 scales; product folded into exp
SPROD = SX * SW                       # 1024

_NC_CACHE = None


def build_nc():
    nc = bacc.Bacc()

    # fp8 DoubleRow-packed inputs: [p, (kt2, i, tok/m)]
    xt8h_d = nc.dram_tensor("xt8h", [128, 4 * TOK], F8, kind="ExternalInput")
    xt8l_d = nc.dram_tensor("xt8l", [128, 4 * TOK], F8, kind="ExternalInput")
    wqk8_d = nc.dram_tensor("wqk8", [128, 4 * 2 * INNER], F8, kind="ExternalInput")
    wv8h_d = nc.dram_tensor("wv8h", [128, 4 * INNER], F8, kind="ExternalInput")
    wv8l_d = nc.dram_tensor("wv8l", [128, 4 * INNER], F8, kind="ExternalInput")
    wout_d = nc.dram_tensor("wout", [INNER, C], BF16, kind="ExternalInput")
    wout2_d = nc.dram_tensor("wout2", [64, 4 * C], BF16, kind="ExternalInput")
    # SVD factors of the bias block (x 2^10 each): X^T Y = B8 * 2^20,
    # duplicated across both partition halves for the two dots row groups
    xf_d = nc.dram_tensor("xf", [128, WTOK], BF16, kind="ExternalInput")
    yf8_d = nc.dram_tensor("yf8", [128, 8 * WTOK], BF16, kind="ExternalInput")
    out_d = nc.dram_tensor("out", [TOK, C], BF16, kind="ExternalOutput")

    K_LAG = int(os.environ.get("K_LAG", "4"))
    K_PSA = int(os.environ.get("K_PSA", "3"))
    K_PSM = int(os.environ.get("K_PSM", "3"))
    K_O0 = int(os.environ.get("K_O0", "14"))
    K_O1 = int(os.environ.get("K_O1", "23"))
    K_LO1 = int(os.environ.get("K_LO1", "19"))
    K_SHIFT = int(os.environ.get("K_SHIFT", "8"))
    QK_DVE = {int(f) for f in os.environ.get("K_QKDVE", "17")}
    K_DR1 = os.environ.get("K_DR1", "0") != "0"
    K_DNE = os.environ.get("K_DNE", "1") != "0"
    K_DR2 = os.environ.get("K_DR2", "0") != "0"
    K_DCA = os.environ.get("K_DCA", "1") != "0"
    K_P0S = os.environ.get("K_P0S", "1") != "0"
    K_DNO = os.environ.get("K_DNO", "0") != "0"
    OP_ACT = {int(f) for f in os.environ.get("K_OPACT", "0246")}
    DUP0 = os.environ.get("K_W0", "0") != "0"
    K_Q4 = os.environ.get("K_Q4", "0") != "0"
    V_DVE = {int(f) for f in os.environ.get("K_VDVE", "")}
    V_SPLIT = {int(f) for f in os.environ.get("K_VSPLIT", "")}
    K_DNE0 = os.environ.get("K_DNE0", "0") != "0"
    K_P0A = int(os.environ.get("K_P0A", "12"))
    K_P0B = int(os.environ.get("K_P0B", "14"))
    K_P1A = int(os.environ.get("K_P1A", "18"))
    K_P1B = int(os.environ.get("K_P1B", "18"))
    K_OF = int(os.environ.get("K_OF", "4"))
    K_MIDF = os.environ.get("K_MIDF", "0") != "0"
    K_P2 = os.environ.get("K_P2", "0") != "0"

    with tile.TileContext(nc) as tc:
        with (
            tc.tile_pool(name="const", bufs=1) as cpool,
            tc.tile_pool(name="sb", bufs=int(os.environ.get("K_SB", "2"))) as sb,
            tc.tile_pool(name="attS", bufs=int(os.environ.get("K_ATTS", "5"))) as attS,
            tc.tile_pool(name="attL", bufs=int(os.environ.get("K_ATTL", "7"))) as attL,
            tc.tile_pool(name="psA", bufs=K_PSA, space="PSUM") as psA,
            tc.tile_pool(name="psD", bufs=1, space="PSUM") as psD,
            tc.tile_pool(name="psM", bufs=K_PSM, space="PSUM") as psM,
        ):
            # ---- PE ramp warm-up anchor + ACT Exp table warm (both memsets
            # must precede the DMA queue stuffing: a memset parked behind
            # slow Pool DMAs blocks the in-order ACT queue at the warm-exp)
            warm1 = cpool.tile([1, WTOK], BF16, tag="warm1")
            nc.gpsimd.memset(warm1[:], 1.0)
            warmps = psD.tile([64, 64], F32, tag="psDA", name="psDA")
            nc.tensor.matmul(warmps[:], warm1[:], warm1[:], start=True, stop=True)
            warm = cpool.tile([1, 2], F32, tag="warm")
            nc.gpsimd.memset(warm[:], 1.0)
            nc.scalar.activation(
                warm[:, 1:2], warm[:, 0:1], mybir.ActivationFunctionType.Exp
            )

            # ---- constants + chunk-0 inputs, interleaved across DMA queues
            wqk8_sb = cpool.tile([128, 2, 2, 2 * INNER], F8, tag="wqk8")
            wv8h_sb = cpool.tile([128, 2, 2, INNER], F8, tag="wv8h")
            wv8l_sb = cpool.tile([128, 2, 2, INNER], F8, tag="wv8l")
            xt8h0 = sb.tile([128, 2, 2, CHUNK], F8, tag="xt8h", name="xt8h")
            xt8l0 = sb.tile([128, 2, 2, CHUNK], F8, tag="xt8l", name="xt8l")

            def xd(d):
                return d.ap().rearrange("p (k i t) -> p k i t", k=2, i=2)

            def wd(d, m):
                return d.ap().rearrange("p (k i m) -> p k i m", k=2, i=2)

            # wave 1: wqk8 + xt8h th0 (first qk groups); wave 2: v operands
            pat = os.environ.get("K_RR", "sgsgsgsgsgsgsgsgsgsgsg")
            emap = {"s": nc.sync, "g": nc.gpsimd, "a": nc.scalar}
            pi = iter(pat)

            def dq():
                return emap[next(pi)]

            if DUP0:
                # first wave: just ft0 (q) + ft4 (k) columns, both kt2 -
                # unblocks the first two qk groups ~300ns earlier
                for kt2 in range(2):
                    for c0 in (0, 512):
                        dq().dma_start(
                            out=wqk8_sb[:, kt2, :, c0 : c0 + 128],
                            in_=wd(wqk8_d, 2 * INNER)[:, kt2, :, c0 : c0 + 128],
                        )
            for kt2 in range(2):
                for mh in range(2):
                    dq().dma_start(
                        out=wqk8_sb[:, kt2, :, mh * 512 + 128 * (mh == 0 and DUP0) : (mh + 1) * 512],
                        in_=wd(wqk8_d, 2 * INNER)[:, kt2, :, mh * 512 + 128 * (mh == 0 and DUP0) : (mh + 1) * 512],
                    )
                for th in range(2):
                    dq().dma_start(
                        out=xt8h0[:, kt2, :, th * 512 : (th + 1) * 512],
                        in_=xd(xt8h_d)[:, kt2, :, th * 512 : (th + 1) * 512],
                    )
            for kt2 in range(2):
                dq().dma_start(out=wv8h_sb[:, kt2], in_=wd(wv8h_d, INNER)[:, kt2])
                dq().dma_start(out=wv8l_sb[:, kt2], in_=wd(wv8l_d, INNER)[:, kt2])
                dq().dma_start(
                    out=xt8l0[:, kt2], in_=xd(xt8l_d)[:, kt2, :, 0:CHUNK]
                )
            # bias factors ride the idle ACT queue (needed by the first
            # dots pair); the slow wout/wout2 setups are deferred into the
            # pipeline (emitted at iteration 2 below) - on the ACT queue at
            # t=0 they delay the first qk copies by ~2.6us
            xf_sb = cpool.tile([128, WTOK], BF16, tag="xf")
            nc.scalar.dma_start(out=xf_sb[:], in_=xf_d.ap())
            yf8_sb = cpool.tile([128, 8 * WTOK], BF16, tag="yf8")
            nc.scalar.dma_start(out=yf8_sb[:], in_=yf8_d.ap())
            wout_sb = cpool.tile([128, 4, C], BF16, tag="wout")
            # duplicate of w_out's odd-head row halves (kt*128+64..kt*128+127)
            # at partitions 0-63 so the drain oproj reads the alo tiles
            # directly instead of waiting on their partition-shift DMAs
            wout2_sb = cpool.tile([64, 4, C], BF16, tag="wout2")

            def load_wout():
                nc.sync.dma_start(
                    out=wout_sb[:],
                    in_=wout_d.ap().rearrange("(kt p) f -> p kt f", p=128),
                )
                nc.gpsimd.dma_start(
                    out=wout2_sb[:],
                    in_=wout2_d.ap().rearrange("p (k c) -> p k c", k=4),
                )

            def load_xt(ch):
                if ch == 0:
                    return xt8h0, xt8l0
                t0 = ch * CHUNK
                xh = sb.tile([128, 2, 2, CHUNK], F8, tag="xt8h", name="xt8h")
                xl = sb.tile([128, 2, 2, CHUNK], F8, tag="xt8l", name="xt8l")
                for kt2 in range(2):
                    eng = nc.sync if kt2 == 0 else nc.gpsimd
                    eng.dma_start(
                        out=xh[:, kt2], in_=xd(xt8h_d)[:, kt2, :, t0 : t0 + CHUNK]
                    )
                    eng2 = nc.gpsimd if kt2 == 0 else nc.sync
                    eng2.dma_start(
                        out=xl[:, kt2], in_=xd(xt8l_d)[:, kt2, :, t0 : t0 + CHUNK]
                    )
                return xh, xl

            DR = mybir.MatmulPerfMode.DoubleRow

            def stage1_groups(xts):
                """24 matmul-group thunks building qkT f-tiles and v tiles."""
                xh, xl = xts
                qk_sb = [
                    sb.tile([128, CHUNK], BF16, tag=f"qk{ft}", name=f"qk{ft}")
                    for ft in range(8)
                ]
                v_sb = [
                    sb.tile([128, HEADS * 65], BF16, tag=f"v{tt}", name=f"v{tt}")
                    for tt in range(CHUNK // 128)
                ]
                vlo_sb = [
                    sb.tile([64, HEADS * 65], BF16, tag=f"vlo{tt}", name=f"vlo{tt}")
                    for tt in range(CHUNK // 128)
                ]
                emitters = []

                def qk_group(ft, th):
                    def emit():
                        ps = psA.tile([128, 512], F32, tag="psA", name="psA")
                        for kt2 in range(2):
                            nc.tensor.matmul(
                                ps[:],
                                wqk8_sb[:, kt2, :, ft * 128 : (ft + 1) * 128],
                                xh[:, kt2, :, th * 512 : (th + 1) * 512],
                                start=(kt2 == 0),
                                stop=(kt2 == 1),
                                perf_mode=DR,
                            )
                        dst = qk_sb[ft][:, th * 512 : (th + 1) * 512]
                        # balance PSUM->SBUF moves: ACT is the loaded engine,
                        # a few late (least-latency-critical) copies go to DVE
                        if th == 1 and ft in QK_DVE:
                            nc.vector.tensor_copy(dst, ps[:])
                        else:
                            nc.scalar.copy(dst, ps[:])

                    return emit

                def v_group(tt):
                    def emit():
                        ps = psA.tile([128, 512], F32, tag="psA", name="psA")
                        first = True
                        for kt2 in range(2):
                            for xa, wa in ((xh, wv8h_sb), (xl, wv8h_sb), (xh, wv8l_sb)):
                                nc.tensor.matmul(
                                    ps[:],
                                    xa[:, kt2, :, tt * 128 : (tt + 1) * 128],
                                    wa[:, kt2],
                                    start=first,
                                    stop=(kt2 == 1 and wa is wv8l_sb),
                                    perf_mode=DR,
                                )
                                first = False
                        vv = v_sb[tt][:].rearrange("p (m c) -> p m c", c=65)
                        if tt in V_SPLIT:
                            # halve the copy: m0-3 on ACT, m4-7 on DVE
                            nc.scalar.mul(
                                vv[:, 0:4, 0:64],
                                ps[:].rearrange("p (m c) -> p m c", c=64)[:, 0:4],
                                1.0 / SPROD,
                            )
                            with nc.allow_low_precision(reason="v scale"):
                                nc.vector.tensor_scalar_mul(
                                    vv[:, 4:8, 0:64],
                                    ps[:].rearrange("p (m c) -> p m c", c=64)[:, 4:8],
                                    1.0 / SPROD,
                                )
                        elif tt in V_DVE:
                            with nc.allow_low_precision(reason="v scale"):
                                nc.vector.tensor_scalar_mul(
                                    vv[:, :, 0:64],
                                    ps[:].rearrange("p (m c) -> p m c", c=64),
                                    1.0 / SPROD,
                                )
                        else:
                            nc.scalar.mul(
                                vv[:, :, 0:64],
                                ps[:].rearrange("p (m c) -> p m c", c=64),
                                1.0 / SPROD,
                            )
                        nc.gpsimd.memset(vv[:, :, 64:65], 1.0)
                        # odd window rows down to 0..63 for mm2
                        nc.sync.dma_start(out=vlo_sb[tt][:], in_=v_sb[tt][64:128, :])

                    return emit

                for ft in range(8):
                    for th in range(CHUNK // 512):
                        emitters.append(qk_group(ft, th))
                for tt in range(CHUNK // 128):
                    emitters.append(v_group(tt))
                return emitters, (qk_sb, v_sb, vlo_sb)

            # ---- one continuous software pipeline across all chunks ----
            chunk_tiles = {}
            chunk_at = {}
            state = {}

            def get_at(ch):
                if ch not in chunk_at:
                    chunk_at[ch] = [
                        sb.tile([128, CHUNK], BF16, tag=f"at{kt}", name=f"at{kt}")
                        for kt in range(4)
                    ]
                return chunk_at[ch]

            EXPSCALE = SCALE / (SPROD * SPROD)

            def emit_front_pair(g):
                # dots for the even/odd head pair (g, g+1): rank-64 bias
                # matmul first (start=True over the whole tile), dots
                # accumulate on top; separate PSUM tiles + tile_position rows
                # per head (same-tile row mixing is a device crash)
                ch, u = divmod(g, NUNITS)
                qk_sb, _, _ = chunk_tiles[ch]
                oct_, m = divmod(u, 8)
                qf = qk_sb[m // 2]
                kf = qk_sb[4 + m // 2]
                dpsA = psD.tile([64, 512], F32, tag="psDA", name="psDA")
                dpsB = psD.tile([64, 512], F32, tag="psDB", name="psDB")
                for dps, hrow in ((dpsA, 0), (dpsB, 64)):
                    nc.tensor.matmul(
                        dps[:],
                        xf_sb[hrow : hrow + 64, :],
                        yf8_sb[hrow : hrow + 64, :],
                        start=True,
                        stop=False,
                        tile_position=(hrow, 0),
                        skip_group_check=True,
                    )
                for nl in range(8):
                    ncol = (oct_ * 8 + nl) * 64
                    for dps, hrow in ((dpsA, 0), (dpsB, 64)):
                        nc.tensor.matmul(
                            dps[:, nl * 64 : (nl + 1) * 64],
                            kf[hrow : hrow + 64, ncol : ncol + 64],
                            qf[hrow : hrow + 64, ncol : ncol + 64],
                            start=False,
                            stop=nl == 7,
                            tile_position=(hrow, 0),
                            skip_group_check=True,
                        )
                for gg, dps in ((g, dpsA), (g + 1, dpsB)):
                    e_t = attL.tile([64, 512], BF16, tag="e", name="e")
                    nc.scalar.activation(
                        e_t[:], dps[:], mybir.ActivationFunctionType.Exp,
                        scale=EXPSCALE,
                    )
                    state[gg] = {"e": e_t, "m": m + (gg - g), "oct": oct_, "ch": ch}

            def emit_mid(g):
                # mm2 (+ones column -> sums row 64), reciprocal, on-chip
                # partition broadcast
                st = state[g]
                m, oct_, e_t, ch = st["m"], st["oct"], st["e"], st["ch"]
                _, v_sb, vlo_sb = chunk_tiles[ch]
                ops = psM.tile([65, 512], F32, tag="psM", name="psM")
                for nl in range(8):
                    tt = oct_ * 4 + nl // 2
                    if nl % 2 == 0:
                        lhsT = v_sb[tt][0:64, m * 65 : (m + 1) * 65]
                    else:
                        lhsT = vlo_sb[tt][:, m * 65 : (m + 1) * 65]
                    nc.tensor.matmul(
                        ops[:, nl * 64 : (nl + 1) * 64],
                        lhsT,
                        e_t[:, nl * 64 : (nl + 1) * 64],
                        start=True,
                        stop=True,
                    )
                r_t = attS.tile([1, 512], BF16, tag="s", name="s")
                with nc.allow_low_precision(reason="softmax recip in bf16"):
                    nc.vector.reciprocal(r_t[:], ops[64:65, :])
                norm = attL.tile([64, 512], BF16, tag="norm", name="norm")
                nc.gpsimd.partition_broadcast(norm[:], r_t[:])
                st["norm"] = norm
                st["ops"] = ops

            drain_alo = {}

            def emit_norm(g):
                # normalize (multiply by 1/sums) + at write
                st = state.pop(g)
                m, oct_, ch = st["m"], st["oct"], st["ch"]
                at_sb = get_at(ch)
                kt = m // 2
                if m % 2 == 0:
                    if K_DNE and ch == NCHUNK - 1 and (oct_ == 1 or K_DNE0):
                        # drain: DVE serializes the last norms while ACT and
                        # Pool idle - stage via ACT, multiply on Pool
                        oo = attL.tile([64, 512], BF16, tag="oo", name="oo")
                        nc.scalar.copy(oo[:], st["ops"][0:64, :])
                        nc.gpsimd.tensor_tensor(
                            at_sb[kt][0:64, oct_ * 512 : (oct_ + 1) * 512],
                            oo[:],
                            st["norm"][:],
                            mybir.AluOpType.mult,
                        )
                    else:
                        nc.vector.tensor_tensor(
                            at_sb[kt][0:64, oct_ * 512 : (oct_ + 1) * 512],
                            st["ops"][0:64, :],
                            st["norm"][:],
                            mybir.AluOpType.mult,
                        )
                else:
                    alo = attL.tile([64, 512], BF16, tag="alo", name="alo")
                    if K_DNO and ch == NCHUNK - 1 and oct_ == 1:
                        oo = attL.tile([64, 512], BF16, tag="oo", name="oo")
                        nc.scalar.copy(oo[:], st["ops"][0:64, :])
                        nc.gpsimd.tensor_tensor(
                            alo[:], oo[:], st["norm"][:], mybir.AluOpType.mult
                        )
                    else:
                        nc.vector.tensor_tensor(
                            alo[:], st["ops"][0:64, :], st["norm"][:],
                            mybir.AluOpType.mult,
                        )
                    if ch == NCHUNK - 1 and (m == 7 or (K_DR1 and m == 5 and oct_ == 1)
                            or (K_DR2 and oct_ == 1)):
                        # drain: oproj reads these tiles directly (via the
                        # wout2 duplicate) - the partition-shift DMAs would
                        # sit on the exit critical path
                        drain_alo[(m, oct_)] = alo
                        return
                    nc.sync.dma_start(
                        out=at_sb[kt][64:128, oct_ * 512 : (oct_ + 1) * 512],
                        in_=alo[:],
                    )

            oproj_part = {}

            def emit_oproj_p0(ch, tt, kts=range(3)):
                # pre-stage kt0..2 before the last at-tile is ready; callable
                # per-kt so the matmul bursts spread across iterations
                at_sb = chunk_at[ch]
                ps = oproj_part.get((ch, tt))
                if ps is None:
                    ps = psA.tile([128, 512], F32, tag="psA", name="psA")
                    oproj_part[(ch, tt)] = ps
                for kt in kts:
                    nc.tensor.matmul(
                        ps[:],
                        at_sb[kt][:, tt * 128 : (tt + 1) * 128],
                        wout_sb[:, kt, :],
                        start=(kt == 0),
                        stop=False,
                        skip_group_check=True,
                    )

            def drain_finish(tt, ps):
                ch = NCHUNK - 1
                t0 = ch * CHUNK + tt * 128
                o_t = attS.tile([128, C], BF16, tag="o", name="o")
                if tt < 6:
                    # all on ACT: DVE's recip/norm chain paces the drain
                    if tt % 2 == 0 or K_DCA:
                        nc.scalar.copy(o_t[:], ps[:])
                    else:
                        nc.vector.tensor_copy(o_t[:], ps[:])
                    eng = nc.sync if tt % 2 == 0 else nc.gpsimd
                    eng.dma_start(out=out_d.ap()[t0 : t0 + 128, :], in_=o_t[:])
                    return
                if tt == 7 and K_Q4:
                    # quarter the final tile: shortest possible exit chain,
                    # last DMA on a HWDGE queue (trailing SWDGE delays exit)
                    engs = ((nc.scalar, nc.sync), (nc.vector, nc.gpsimd),
                            (nc.scalar, nc.sync), (nc.vector, nc.scalar))
                    for q, (ce, de) in enumerate(engs):
                        c0 = q * 128
                        if ce is nc.vector:
                            nc.vector.tensor_copy(
                                o_t[:, c0 : c0 + 128], ps[:, c0 : c0 + 128]
                            )
                        else:
                            nc.scalar.copy(
                                o_t[:, c0 : c0 + 128], ps[:, c0 : c0 + 128]
                            )
                        de.dma_start(
                            out=out_d.ap()[t0 : t0 + 128, c0 : c0 + 128],
                            in_=o_t[:, c0 : c0 + 128],
                        )
                    return
                nc.scalar.copy(o_t[:, 0:256], ps[:, 0:256])
                nc.vector.tensor_copy(o_t[:, 256:512], ps[:, 256:512])
                nc.sync.dma_start(
                    out=out_d.ap()[t0 : t0 + 128, 0:256], in_=o_t[:, 0:256]
                )
                eng2 = nc.scalar if tt == 7 else nc.gpsimd
                eng2.dma_start(
                    out=out_d.ap()[t0 : t0 + 128, 256:512], in_=o_t[:, 256:512]
                )

            def emit_oproj_drain0(tt):
                # last chunk oct0: kt0-2 prestaged full (its alo DMAs land in
                # time); kt3 split so only the m7 alo tile is read directly
                ch = NCHUNK - 1
                at_sb = chunk_at[ch]
                ps = oproj_part.pop((ch, tt))
                nc.tensor.matmul(
                    ps[:],
                    at_sb[3][0:64, tt * 128 : (tt + 1) * 128],
                    wout_sb[0:64, 3, :],
                    start=False,
                    stop=False,
                    skip_group_check=True,
                )
                nc.tensor.matmul(
                    ps[:],
                    drain_alo[(7, 0)][:, (tt % 4) * 128 :][:, 0:128],
                    wout2_sb[:, 3, :],
                    start=False,
                    stop=True,
                    skip_group_check=True,
                )
                drain_finish(tt, ps)

            def emit_oproj_p01(tt):
                # oct1 prestage: full kt matmuls for those whose alo DMAs
                # land in time (kt0-1 under K_DR1, kt0-2 otherwise)
                ch = NCHUNK - 1
                at_sb = chunk_at[ch]
                ps = psA.tile([128, 512], F32, tag="psA", name="psA")
                for kt in range(2 if K_DR1 else 3):
                    nc.tensor.matmul(
                        ps[:],
                        at_sb[kt][:, tt * 128 : (tt + 1) * 128],
                        wout_sb[:, kt, :],
                        start=(kt == 0),
                        stop=False,
                        skip_group_check=True,
                    )
                oproj_part[(ch, tt)] = ps

            dr2_ps = {}

            def emit_dr2(kt):
                # full alo-direct oct1: stage kt's lo+hi K=64 halves across
                # all 4 tiles as their norm outputs arrive - zero DMA waits
                ch = NCHUNK - 1
                at_sb = chunk_at[ch]
                tts = [4, 5, 6, 7]
                if kt == 0:
                    for tt in tts:
                        dr2_ps[tt] = psA.tile(
                            [128, 512], F32, tag="psA", name="psA"
                        )
                for tt in tts:
                    nc.tensor.matmul(
                        dr2_ps[tt][:],
                        at_sb[kt][0:64, tt * 128 : (tt + 1) * 128],
                        wout_sb[0:64, kt, :],
                        start=(kt == 0),
                        stop=False,
                        skip_group_check=True,
                    )
                alo = drain_alo[(2 * kt + 1, 1)]
                for tt in tts:
                    c0 = (tt % 4) * 128
                    nc.tensor.matmul(
                        dr2_ps[tt][:],
                        alo[:, c0 : c0 + 128],
                        wout2_sb[:, kt, :],
                        start=False,
                        stop=(kt == 3),
                        skip_group_check=True,
                    )
                if kt == 3:
                    for tt in tts:
                        drain_finish(tt, dr2_ps.pop(tt))

            def emit_oproj_drain1():
                # oct1 finals: kt2/kt3 split, kt-major within tile pairs so
                # the in-order PE queue consumes operands in norm-arrival
                # order (m4, m5, m6, m7); nothing else needs the PE after
                ch = NCHUNK - 1
                at_sb = chunk_at[ch]
                tts = [4, 5, 6, 7]
                pss = {tt: oproj_part.pop((ch, tt)) for tt in tts}
                for kt in ((2, 3) if K_DR1 else (3,)):
                    for tt in tts:
                        nc.tensor.matmul(
                            pss[tt][:],
                            at_sb[kt][0:64, tt * 128 : (tt + 1) * 128],
                            wout_sb[0:64, kt, :],
                            start=False,
                            stop=False,
                            skip_group_check=True,
                        )
                    alo = drain_alo[(2 * kt + 1, 1)]
                    for tt in tts:
                        c0 = (tt % 4) * 128
                        nc.tensor.matmul(
                            pss[tt][:],
                            alo[:, c0 : c0 + 128],
                            wout2_sb[:, kt, :],
                            start=False,
                            stop=(kt == 3),
                            skip_group_check=True,
                        )
                for tt in tts:
                    drain_finish(tt, pss[tt])

            def emit_oproj(ch, tt):
                at_sb = chunk_at[ch]
                ps = oproj_part.pop((ch, tt), None)
                if ps is not None:
                    nc.tensor.matmul(
                        ps[:],
                        at_sb[3][:, tt * 128 : (tt + 1) * 128],
                        wout_sb[:, 3, :],
                        start=False,
                        stop=True,
                        skip_group_check=True,
                    )
                else:
                    ps = psA.tile([128, 512], F32, tag="psA", name="psA")
                    for kt in range(4):
                        nc.tensor.matmul(
                            ps[:],
                            at_sb[kt][:, tt * 128 : (tt + 1) * 128],
                            wout_sb[:, kt, :],
                            start=(kt == 0),
                            stop=(kt == 3),
                        )
                t0 = ch * CHUNK + tt * 128
                # PSUM -> SBUF split across ACT/DVE (gpsimd cannot read PSUM)
                o_t = attS.tile([128, C], BF16, tag="o", name="o")
                if ch == NCHUNK - 1 and tt >= 6:
                    # final tiles: halve the copy across ACT+DVE and the out
                    # DMA across SP+Pool so the exit chain shortens
                    nc.scalar.copy(o_t[:, 0:256], ps[:, 0:256])
                    nc.vector.tensor_copy(o_t[:, 256:512], ps[:, 256:512])
                    nc.sync.dma_start(
                        out=out_d.ap()[t0 : t0 + 128, 0:256], in_=o_t[:, 0:256]
                    )
                    eng2 = nc.scalar if tt == 7 else nc.gpsimd
                    eng2.dma_start(
                        out=out_d.ap()[t0 : t0 + 128, 256:512], in_=o_t[:, 256:512]
                    )
                    return
                if tt in OP_ACT:
                    nc.scalar.copy(o_t[:], ps[:])
                else:
                    nc.vector.tensor_copy(o_t[:], ps[:])
                eng = nc.sync if tt % 2 == 0 else nc.gpsimd
                eng.dma_start(out=out_d.ap()[t0 : t0 + 128, :], in_=o_t[:])

            # ---- stage-1 work scheduled just-in-time ----
            NG = NCHUNK * NUNITS
            PRE = 4
            sched = {}

            def at_iter(i, fn):
                sched.setdefault(i, []).append(fn)

            group_order = []
            for th in range(2):
                for ft in (0, 4, 1, 5, 2, 6, 3, 7):
                    group_order.append(ft * 2 + th)
                group_order.extend(16 + th * 4 + tt for tt in range(4))
            SPREADS = {
                # 2 groups/iter bursts (original)
                "a": [0, 0, 1, 1, 2, 2, 4, 4, 3, 3, 5, 5,
                      8, 8, 9, 9, 10, 10, 11, 11, 12, 12, 13, 13],
                # flat: 1 qk group/iter so the ACT copy stream never bursts
                "f": [0, 1, 2, 3, 4, 5, 6, 7,
                      8, 8, 9, 9,
                      9, 10, 11, 12, 13, 14, 15, 16,
                      16, 16, 17, 17],
                "g": [0, 1, 2, 3, 4, 5, 6, 7,
                      7, 8, 8, 9,
                      9, 10, 11, 12, 13, 14, 15, 16,
                      15, 16, 17, 17],
            }
            spread = SPREADS[os.environ.get("K_SPREAD", "a")]

            group_order0 = (
                [ft * 2 + 0 for ft in (0, 4, 1, 5, 2, 6, 3, 7)]
                + [16, 17, 18, 19]
                + [ft * 2 + 1 for ft in (0, 4, 1, 5, 2, 6, 3, 7)]
                + [20, 21, 22, 23]
            )
            spread0 = [-4, -4, -3, -3, -2, -2, -1, -1,
                       0, 0, 1, 1,
                       2, 2, 3, 3, 4, 4, 5, 5,
                       6, 6, 7, 7]

            def schedule_chunk(ch):
                emitters, tiles = stage1_groups(load_xt(ch))
                chunk_tiles[ch] = tiles
                if ch == 0:
                    order, offs, base = group_order0, spread0, 0
                else:
                    order, offs, base = (
                        group_order, [o + K_SHIFT for o in spread],
                        (ch - 1) * NUNITS,
                    )
                for j, gi in enumerate(order):
                    at_iter(base + offs[j], emitters[gi])

            for i in range(-PRE, NG + 24):
                if i == -PRE:
                    schedule_chunk(0)
                if i == 2:
                    load_wout()
                for ch in range(1, NCHUNK):
                    if i == (ch - 1) * NUNITS:
                        schedule_chunk(ch)
                for fn in sched.get(i, []):
                    fn()
                if K_MIDF:
                    # mm2 first: unit i-1's mm2 jumps ahead of pair i's
                    # dots in the in-order PE queue
                    if 0 <= i - 1 < NG:
                        emit_mid(i - 1)
                    if 0 <= i < NG and i % 2 == 0:
                        emit_front_pair(i)
                else:
                    if 0 <= i < NG and i % 2 == 0:
                        emit_front_pair(i)
                    if 0 <= i - 1 < NG:
                        emit_mid(i - 1)
                gg = i - K_LAG
                if 0 <= gg < NG:
                    if gg >= NG - K_OF:
                        # drain: odd head first so the partition-shift DMA
                        # (alo) starts earlier; both norms of the pair at once
                        if gg % 2 == 0:
                            emit_norm(gg + 1)
                            emit_norm(gg)
                    else:
                        emit_norm(gg)
                for ch in range(NCHUNK - 1):
                    cb = ch * NUNITS
                    if K_P2 and ch in (1, 2):
                        # prestage kt0-2 of the o1 wave so only the cheap
                        # kt3 finals land in the next chunk's dots window
                        if i == cb + K_O1 - 2:
                            emit_oproj_p0(ch, 4)
                            emit_oproj_p0(ch, 5)
                        if i == cb + K_O1 - 1:
                            emit_oproj_p0(ch, 6)
                            emit_oproj_p0(ch, 7)
                    if cb + K_O0 <= i <= cb + K_O0 + 3:
                        emit_oproj(ch, i - cb - K_O0)
                    if cb + K_O1 <= i <= cb + K_O1 + 3:
                        emit_oproj(ch, i - cb - K_O1 + 4)
                cb3 = (NCHUNK - 1) * NUNITS
                if cb3 + K_O0 <= i <= cb3 + K_O0 + 3:
                    emit_oproj_drain0(i - cb3 - K_O0)
                if K_P0S:
                    for kt in range(3):
                        if i == cb3 + K_P0A + kt - 2:
                            emit_oproj_p0(NCHUNK - 1, 0, [kt])
                            emit_oproj_p0(NCHUNK - 1, 1, [kt])
                        if i == cb3 + K_P0B + kt - 2:
                            emit_oproj_p0(NCHUNK - 1, 2, [kt])
                            emit_oproj_p0(NCHUNK - 1, 3, [kt])
                else:
                    if i == cb3 + K_P0A:
                        emit_oproj_p0(NCHUNK - 1, 0)
                        emit_oproj_p0(NCHUNK - 1, 1)
                    if i == cb3 + K_P0B:
                        emit_oproj_p0(NCHUNK - 1, 2)
                        emit_oproj_p0(NCHUNK - 1, 3)
                if K_DR2:
                    if cb3 + K_LO1 - 3 <= i <= cb3 + K_LO1:
                        emit_dr2(i - cb3 - K_LO1 + 3)
                else:
                    if i == cb3 + K_P1A:
                        emit_oproj_p01(4)
                        emit_oproj_p01(5)
                    if i == cb3 + K_P1B:
                        emit_oproj_p01(6)
                        emit_oproj_p01(7)
                    if i == cb3 + K_LO1:
                        emit_oproj_drain1()

    nc.finalize()
    return nc


def _get_nc():
    global _NC_CACHE
    if _NC_CACHE is None:
        _NC_CACHE = build_nc()
    return _NC_CACHE


def _bias_factors(pos_emb: np.ndarray):
    """B8*2^20 = X^T Y (factors x2^10 each), duplicated across row halves."""
    idx = np.array([[i, j] for i in range(WS) for j in range(WS)])
    rel = idx[None, :, :] - idx[:, None, :] + WS - 1
    bias = pos_emb[rel[:, :, 0], rel[:, :, 1]]            # [q, k]
    b8 = bias.T.astype(np.float64) * 8.0                  # [k, q] * 1/SCALE
    u, s, vt = np.linalg.svd(b8)
    x = (u * np.sqrt(s)).T * SPROD                        # [r=64, k=64]
    y = (np.sqrt(s)[:, None] * vt) * SPROD                # [r=64, q=64]
    xf = np.tile(x, (2, 1)).astype(ml_dtypes.bfloat16)    # [128, 64]
    yf8 = np.tile(np.tile(y, (1, 8)), (2, 1)).astype(ml_dtypes.bfloat16)
    return np.ascontiguousarray(xf), np.ascontiguousarray(yf8)


def _pack8(a, rows):
    """[512, n] -> fp8 [128, 4*n] with c = 256*kt2 + 128*i + p."""
    n = a.shape[1]
    a = a.reshape(2, 2, 128, n).transpose(2, 0, 1, 3).reshape(128, 4 * n)
    return np.ascontiguousarray(a.astype(NPF8))


def host_prep(x, w_qkv, pos_emb, w_out, b_out):
    """Shard + quantize + lay out the inputs: one in_map per core."""
    x = np.asarray(x, dtype=np.float32)
    w_qkv = np.asarray(w_qkv, dtype=np.float32)
    pos_emb = np.asarray(pos_emb, dtype=np.float32)
    w_out = np.ascontiguousarray(np.asarray(w_out, dtype=np.float32))

    nh = H // WS
    # [slice, c, tok'] with tok' in window order (nh, nw, wsh, wsw)
    xt = x.reshape(B * L, nh, WS, nh, WS, C).transpose(0, 5, 1, 3, 2, 4)
    xt = np.ascontiguousarray(xt.reshape(B * L, C, TOK)) * SX
    xh8 = xt.astype(NPF8)
    xl8 = (xt - xh8.astype(np.float32)).astype(NPF8)

    wqk = w_qkv[:, : 2 * INNER] * SW
    wv = w_qkv[:, 2 * INNER :] * SW
    wv8h = wv.astype(NPF8)
    wv8l = (wv - wv8h.astype(np.float32)).astype(NPF8)

    wqk8 = _pack8(wqk, 512)
    wv8h_p = _pack8(wv8h.astype(np.float32), 512)
    wv8l_p = _pack8(wv8l.astype(np.float32), 512)
    wout = np.ascontiguousarray(w_out.astype(ml_dtypes.bfloat16))
    wout2 = w_out.reshape(4, 128, C)[:, 64:128].transpose(1, 0, 2)
    wout2 = np.ascontiguousarray(
        wout2.reshape(64, 4 * C).astype(ml_dtypes.bfloat16)
    )
    xf, yf8 = _bias_factors(pos_emb)

    maps = []
    for s in range(NCORES):
        maps.append({
            "xt8h": _pack8(xh8[s].astype(np.float32), TOK),
            "xt8l": _pack8(xl8[s].astype(np.float32), TOK),
            "wqk8": wqk8,
            "wv8h": wv8h_p,
            "wv8l": wv8l_p,
            "wout": wout,
            "wout2": wout2,
            "xf": xf,
            "yf8": yf8,
        })
    return maps


def host_post(out_slices, b_out):
    """[NCORES x (tok', c)] window-ordered -> [b, l, h, w, c] (+ b_out)."""
    nh = H // WS
    out = np.stack([np.asarray(o) for o in out_slices]).astype(np.float32)
    out = out.reshape(B * L, nh, nh, WS, WS, C).transpose(0, 1, 3, 2, 4, 5)
    out = out.reshape(B, L, H, W, C)
    return np.ascontiguousarray(out + np.asarray(b_out, dtype=np.float32))


def kernel(x, w_qkv, pos_emb, w_out, b_out):
    in_maps = host_prep(x, w_qkv, pos_emb, w_out, b_out)
    nc = _get_nc()
    res = run_bass_kernel_spmd(nc, in_maps, list(range(NCORES)))
    return host_post([res.results[s]["out"] for s in range(NCORES)], b_out)


# revision 68
# speedup vs baseline: 1.0207x; 1.0207x over previous
"""Window attention (BaseWindowAttention) Trainium2 kernel, v2.

Data-parallel over the 8 (b,l) slices, one NeuronCore each. Host prep:
transpose each slice to [c, tok] with tokens in window order, quantize to
fp8e4 (x split hi+lo at scale 16, weights at scale 64) packed for DoubleRow
matmuls: channel c = 256*kt2 + 128*i + p lives at [partition p, pair slot i]
of k-tile-pair kt2. Cost notes: PE matmul time = out-free-size x cycles/row
(bf16 1.0, fp8 DoubleRow 0.5, independent of K), so fp8 DR quarters the
qk-projection PE time and rank-64 bias accumulate rides free in K.

Device pipeline per chunk (1024 tokens = 2 octs of 8 windows):
  stage 1: qk projection = 1-pass fp8 DoubleRow (2 matmuls per f-tile-half,
           ~1.1% extra rel err, total 1.24e-2 vs 2e-2 gate); v projection =
           3-pass hi/lo-compensated fp8 DoubleRow (exact to ~bf16); ones
           column memset for the softmax denominators. PSUM->SBUF copies
           balance ACT (most) vs DVE (K_QKDVE f-tiles at th1).
  dots:    per even/odd head pair, the relative-position bias enters PSUM
           first via one rank-64 SVD matmul per 64-row half (factors
           pre-scaled by 2^10 each to match the fp8 scale product 2^20),
           dots accumulate on top (no DVE bias adds); exp on ACT with
           scale SCALE/2^20.
  mm2:     ones-column appended to v puts the denominators in PSUM row 64;
           DVE reciprocal -> gpsimd partition_broadcast (on-chip, no DRAM
           round trip); DVE tensor_tensor mult normalizes into the at
           tiles (odd head via an SP-queue SBUF DMA for the partition
           shift).
  oproj:   bf16 matmuls; PSUM->SBUF copies split ACT/DVE, out DMAs on
           SP/Pool.

Drain (last chunk): odd-head norms for m7 (and optionally m5-oct1) skip
their shift DMA - the oproj reads those alo tiles directly via wout2, a
host-side duplicate of w_out's odd-head row halves at partitions 0-63,
with the kt3 matmul split into two K=64 halves. Even-head norms of the
final oct run as ACT copy + Pool multiply (DVE's recip chain paces the
drain). kt0-2 are prestaged per-kt; the final copies/out-DMAs split
across ACT+DVE / SP+Pool+ACT queues.

Startup: PE-ramp warmup matmul + ACT Exp-table warm (their memsets must
precede the DMA emissions or the in-order ACT queue stalls); bias factors
ride the idle ACT queue; the slow wout/wout2 DMA setups (~1.6us each) are
deferred to pipeline iteration 2 on SP/Pool - on the ACT queue at t=0
they delay the first qk copies by ~2.6us.

Engine budget (cost model, per core): ACT ~110us (exp + copies) is the
pacer at 92% busy; PE ~105us; DVE ~96us; SP/Pool ~50us.

Backend landmines (bisected): column tile_position crashes the device;
mixing tile_position rows within one PSUM tile crashes the device; AluOp
divide does not compile; tensor_tensor cannot read two PSUM operands;
GPSIMD cannot access PSUM (BIR verifier); DMA cannot read PSUM;
partition-stride-0 APs are rejected outside DMA/partition_broadcast.

Self-contained: shapes hardcoded, no sibling imports.
"""
import os
import numpy as np
import ml_dtypes

import concourse.mybir as mybir
import concourse.tile as tile
from concourse import bacc
from concourse.bass_utils import run_bass_kernel_spmd

F32 = mybir.dt.float32
BF16 = mybir.dt.bfloat16
F8 = mybir.dt.float8e4
NPF8 = ml_dtypes.float8_e4m3

B, L, H, W, C = 2, 4, 64, 64, 512
HEADS, CH, WS = 8, 64, 8
WTOK = WS * WS                        # 64 tokens per window
TOK = H * W                           # 4096 tokens per slice
INNER = HEADS * CH                    # 512
SCALE = CH ** -0.5                    # 0.125
CHUNK = 1024                          # tokens per pipeline chunk (16 windows)
NCHUNK = TOK // CHUNK                 # 4
NUNITS = 16                           # attention units per chunk (8 heads x 2)
NCORES = 8
SX, SW = 16.0, 64.0                   # fp8 scales; product folded into exp
SPROD = SX * SW                       # 1024

_NC_CACHE = None


def build_nc():
    nc = bacc.Bacc()

    # fp8 DoubleRow-packed inputs: [p, (kt2, i, tok/m)]
    xt8h_d = nc.dram_tensor("xt8h", [128, 4 * TOK], F8, kind="ExternalInput")
    xt8l_d = nc.dram_tensor("xt8l", [128, 4 * TOK], F8, kind="ExternalInput")
    wqk8_d = nc.dram_tensor("wqk8", [128, 4 * 2 * INNER], F8, kind="ExternalInput")
    wv8h_d = nc.dram_tensor("wv8h", [128, 4 * INNER], F8, kind="ExternalInput")
    wv8l_d = nc.dram_tensor("wv8l", [128, 4 * INNER], F8, kind="ExternalInput")
    wout_d = nc.dram_tensor("wout", [INNER, C], BF16, kind="ExternalInput")
    wout2_d = nc.dram_tensor("wout2", [64, 4 * C], BF16, kind="ExternalInput")
    # SVD factors of the bias block (x 2^10 each): X^T Y = B8 * 2^20,
    # duplicated across both partition halves for the two dots row groups
    xf_d = nc.dram_tensor("xf", [128, WTOK], BF16, kind="ExternalInput")
    yf8_d = nc.dram_tensor("yf8", [128, 8 * WTOK], BF16, kind="ExternalInput")
    out_d = nc.dram_tensor("out", [TOK, C], BF16, kind="ExternalOutput")

    K_LAG = int(os.environ.get("K_LAG", "4"))
    K_PSA = int(os.environ.get("K_PSA", "3"))
    K_PSM = int(os.environ.get("K_PSM", "3"))
    K_O0 = int(os.environ.get("K_O0", "14"))
    K_O1 = int(os.environ.get("K_O1", "23"))
    K_LO1 = int(os.environ.get("K_LO1", "19"))
    K_SHIFT = int(os.environ.get("K_SHIFT", "8"))
    QK_DVE = {int(f) for f in os.environ.get("K_QKDVE", "17")}
    K_DR1 = os.environ.get("K_DR1", "0") != "0"
    K_DNE = os.environ.get("K_DNE", "1") != "0"
    K_DR2 = os.environ.get("K_DR2", "0") != "0"
    K_DCA = os.environ.get("K_DCA", "1") != "0"
    K_P0S = os.environ.get("K_P0S", "1") != "0"
    K_DNO = os.environ.get("K_DNO", "0") != "0"
    OP_ACT = {int(f) for f in os.environ.get("K_OPACT", "0246")}
    DUP0 = os.environ.get("K_W0", "0") != "0"
    K_Q4 = os.environ.get("K_Q4", "0") != "0"
    V_DVE = {int(f) for f in os.environ.get("K_VDVE", "")}
    V_SPLIT = {int(f) for f in os.environ.get("K_VSPLIT", "")}
    K_DNE0 = os.environ.get("K_DNE0", "0") != "0"
    K_P0A = int(os.environ.get("K_P0A", "12"))
    K_P0B = int(os.environ.get("K_P0B", "14"))
    K_P1A = int(os.environ.get("K_P1A", "18"))
    K_P1B = int(os.environ.get("K_P1B", "18"))
    K_OF = int(os.environ.get("K_OF", "4"))
    K_MIDF = os.environ.get("K_MIDF", "0") != "0"
    K_P2 = os.environ.get("K_P2", "0") != "0"
    C0D_SET = {int(f) for f in os.environ.get("K_C0D", "1452637")}

    with tile.TileContext(nc) as tc:
        with (
            tc.tile_pool(name="const", bufs=1) as cpool,
            tc.tile_pool(name="sb", bufs=int(os.environ.get("K_SB", "2"))) as sb,
            tc.tile_pool(name="attS", bufs=int(os.environ.get("K_ATTS", "5"))) as attS,
            tc.tile_pool(name="attL", bufs=int(os.environ.get("K_ATTL", "7"))) as attL,
            tc.tile_pool(name="psA", bufs=K_PSA, space="PSUM") as psA,
            tc.tile_pool(name="psD", bufs=1, space="PSUM") as psD,
            tc.tile_pool(name="psM", bufs=K_PSM, space="PSUM") as psM,
        ):
            # ---- PE ramp warm-up anchor + ACT Exp table warm (both memsets
            # must precede the DMA queue stuffing: a memset parked behind
            # slow Pool DMAs blocks the in-order ACT queue at the warm-exp)
            warm1 = cpool.tile([1, WTOK], BF16, tag="warm1")
            nc.gpsimd.memset(warm1[:], 1.0)
            warmps = psD.tile([64, 64], F32, tag="psDA", name="psDA")
            nc.tensor.matmul(warmps[:], warm1[:], warm1[:], start=True, stop=True)
            warm = cpool.tile([1, 2], F32, tag="warm")
            nc.gpsimd.memset(warm[:], 1.0)
            nc.scalar.activation(
                warm[:, 1:2], warm[:, 0:1], mybir.ActivationFunctionType.Exp
            )

            # ---- constants + chunk-0 inputs, interleaved across DMA queues
            wqk8_sb = cpool.tile([128, 2, 2, 2 * INNER], F8, tag="wqk8")
            wv8h_sb = cpool.tile([128, 2, 2, INNER], F8, tag="wv8h")
            wv8l_sb = cpool.tile([128, 2, 2, INNER], F8, tag="wv8l")
            xt8h0 = sb.tile([128, 2, 2, CHUNK], F8, tag="xt8h", name="xt8h")
            xt8l0 = sb.tile([128, 2, 2, CHUNK], F8, tag="xt8l", name="xt8l")

            def xd(d):
                return d.ap().rearrange("p (k i t) -> p k i t", k=2, i=2)

            def wd(d, m):
                return d.ap().rearrange("p (k i m) -> p k i m", k=2, i=2)

            # wave 1: wqk8 + xt8h th0 (first qk groups); wave 2: v operands
            pat = os.environ.get("K_RR", "sgsgsgsgsgsgsgsgsgsgsg")
            emap = {"s": nc.sync, "g": nc.gpsimd, "a": nc.scalar}
            pi = iter(pat)

            def dq():
                return emap[next(pi)]

            if DUP0:
                # first wave: just ft0 (q) + ft4 (k) columns, both kt2 -
                # unblocks the first two qk groups ~300ns earlier
                for kt2 in range(2):
                    for c0 in (0, 512):
                        dq().dma_start(
                            out=wqk8_sb[:, kt2, :, c0 : c0 + 128],
                            in_=wd(wqk8_d, 2 * INNER)[:, kt2, :, c0 : c0 + 128],
                        )
            for kt2 in range(2):
                for mh in range(2):
                    dq().dma_start(
                        out=wqk8_sb[:, kt2, :, mh * 512 + 128 * (mh == 0 and DUP0) : (mh + 1) * 512],
                        in_=wd(wqk8_d, 2 * INNER)[:, kt2, :, mh * 512 + 128 * (mh == 0 and DUP0) : (mh + 1) * 512],
                    )
                for th in range(2):
                    dq().dma_start(
                        out=xt8h0[:, kt2, :, th * 512 : (th + 1) * 512],
                        in_=xd(xt8h_d)[:, kt2, :, th * 512 : (th + 1) * 512],
                    )
            for kt2 in range(2):
                dq().dma_start(out=wv8h_sb[:, kt2], in_=wd(wv8h_d, INNER)[:, kt2])
                dq().dma_start(out=wv8l_sb[:, kt2], in_=wd(wv8l_d, INNER)[:, kt2])
                dq().dma_start(
                    out=xt8l0[:, kt2], in_=xd(xt8l_d)[:, kt2, :, 0:CHUNK]
                )
            # bias factors ride the idle ACT queue (needed by the first
            # dots pair); the slow wout/wout2 setups are deferred into the
            # pipeline (emitted at iteration 2 below) - on the ACT queue at
            # t=0 they delay the first qk copies by ~2.6us
            xf_sb = cpool.tile([128, WTOK], BF16, tag="xf")
            nc.scalar.dma_start(out=xf_sb[:], in_=xf_d.ap())
            yf8_sb = cpool.tile([128, 8 * WTOK], BF16, tag="yf8")
            nc.scalar.dma_start(out=yf8_sb[:], in_=yf8_d.ap())
            wout_sb = cpool.tile([128, 4, C], BF16, tag="wout")
            # duplicate of w_out's odd-head row halves (kt*128+64..kt*128+127)
            # at partitions 0-63 so the drain oproj reads the alo tiles
            # directly instead of waiting on their partition-shift DMAs
            wout2_sb = cpool.tile([64, 4, C], BF16, tag="wout2")

            def load_wout():
                nc.sync.dma_start(
                    out=wout_sb[:],
                    in_=wout_d.ap().rearrange("(kt p) f -> p kt f", p=128),
                )
                nc.gpsimd.dma_start(
                    out=wout2_sb[:],
                    in_=wout2_d.ap().rearrange("p (k c) -> p k c", k=4),
                )

            def load_xt(ch):
                if ch == 0:
                    return xt8h0, xt8l0
                t0 = ch * CHUNK
                xh = sb.tile([128, 2, 2, CHUNK], F8, tag="xt8h", name="xt8h")
                xl = sb.tile([128, 2, 2, CHUNK], F8, tag="xt8l", name="xt8l")
                for kt2 in range(2):
                    eng = nc.sync if kt2 == 0 else nc.gpsimd
                    eng.dma_start(
                        out=xh[:, kt2], in_=xd(xt8h_d)[:, kt2, :, t0 : t0 + CHUNK]
                    )
                    eng2 = nc.gpsimd if kt2 == 0 else nc.sync
                    eng2.dma_start(
                        out=xl[:, kt2], in_=xd(xt8l_d)[:, kt2, :, t0 : t0 + CHUNK]
                    )
                return xh, xl

            DR = mybir.MatmulPerfMode.DoubleRow

            def stage1_groups(xts, ch=1):
                """24 matmul-group thunks building qkT f-tiles and v tiles."""
                xh, xl = xts
                qk_sb = [
                    sb.tile([128, CHUNK], BF16, tag=f"qk{ft}", name=f"qk{ft}")
                    for ft in range(8)
                ]
                v_sb = [
                    sb.tile([128, HEADS * 65], BF16, tag=f"v{tt}", name=f"v{tt}")
                    for tt in range(CHUNK // 128)
                ]
                vlo_sb = [
                    sb.tile([64, HEADS * 65], BF16, tag=f"vlo{tt}", name=f"vlo{tt}")
                    for tt in range(CHUNK // 128)
                ]
                emitters = []

                def qk_group(ft, th):
                    def emit():
                        ps = psA.tile([128, 512], F32, tag="psA", name="psA")
                        for kt2 in range(2):
                            nc.tensor.matmul(
                                ps[:],
                                wqk8_sb[:, kt2, :, ft * 128 : (ft + 1) * 128],
                                xh[:, kt2, :, th * 512 : (th + 1) * 512],
                                start=(kt2 == 0),
                                stop=(kt2 == 1),
                                perf_mode=DR,
                            )
                        dst = qk_sb[ft][:, th * 512 : (th + 1) * 512]
                        # balance PSUM->SBUF moves: ACT is the loaded engine,
                        # a few late (least-latency-critical) copies go to DVE
                        if (th == 1 and ft in QK_DVE) or (
                            ch == 0 and th == 0 and ft in C0D_SET
                        ):
                            # chunk 0: DVE is idle at startup - parallel
                            # first copies pull the whole ACT stream earlier
                            nc.vector.tensor_copy(dst, ps[:])
                        else:
                            nc.scalar.copy(dst, ps[:])

                    return emit

                def v_group(tt):
                    def emit():
                        ps = psA.tile([128, 512], F32, tag="psA", name="psA")
                        first = True
                        for kt2 in range(2):
                            for xa, wa in ((xh, wv8h_sb), (xl, wv8h_sb), (xh, wv8l_sb)):
                                nc.tensor.matmul(
                                    ps[:],
                                    xa[:, kt2, :, tt * 128 : (tt + 1) * 128],
                                    wa[:, kt2],
                                    start=first,
                                    stop=(kt2 == 1 and wa is wv8l_sb),
                                    perf_mode=DR,
                                )
                                first = False
                        vv = v_sb[tt][:].rearrange("p (m c) -> p m c", c=65)
                        if tt in V_SPLIT:
                            # halve the copy: m0-3 on ACT, m4-7 on DVE
                            nc.scalar.mul(
                                vv[:, 0:4, 0:64],
                                ps[:].rearrange("p (m c) -> p m c", c=64)[:, 0:4],
                                1.0 / SPROD,
                            )
                            with nc.allow_low_precision(reason="v scale"):
                                nc.vector.tensor_scalar_mul(
                                    vv[:, 4:8, 0:64],
                                    ps[:].rearrange("p (m c) -> p m c", c=64)[:, 4:8],
                                    1.0 / SPROD,
                                )
                        elif tt in V_DVE:
                            with nc.allow_low_precision(reason="v scale"):
                                nc.vector.tensor_scalar_mul(
                                    vv[:, :, 0:64],
                                    ps[:].rearrange("p (m c) -> p m c", c=64),
                                    1.0 / SPROD,
                                )
                        else:
                            nc.scalar.mul(
                                vv[:, :, 0:64],
                                ps[:].rearrange("p (m c) -> p m c", c=64),
                                1.0 / SPROD,
                            )
                        nc.gpsimd.memset(vv[:, :, 64:65], 1.0)
                        # odd window rows down to 0..63 for mm2
                        nc.sync.dma_start(out=vlo_sb[tt][:], in_=v_sb[tt][64:128, :])

                    return emit

                for ft in range(8):
                    for th in range(CHUNK // 512):
                        emitters.append(qk_group(ft, th))
                for tt in range(CHUNK // 128):
                    emitters.append(v_group(tt))
                return emitters, (qk_sb, v_sb, vlo_sb)

            # ---- one continuous software pipeline across all chunks ----
            chunk_tiles = {}
            chunk_at = {}
            state = {}

            def get_at(ch):
                if ch not in chunk_at:
                    chunk_at[ch] = [
                        sb.tile([128, CHUNK], BF16, tag=f"at{kt}", name=f"at{kt}")
                        for kt in range(4)
                    ]
                return chunk_at[ch]

            EXPSCALE = SCALE / (SPROD * SPROD)

            def emit_front_pair(g):
                # dots for the even/odd head pair (g, g+1): rank-64 bias
                # matmul first (start=True over the whole tile), dots
                # accumulate on top; separate PSUM tiles + tile_position rows
                # per head (same-tile row mixing is a device crash)
                ch, u = divmod(g, NUNITS)
                qk_sb, _, _ = chunk_tiles[ch]
                oct_, m = divmod(u, 8)
                qf = qk_sb[m // 2]
                kf = qk_sb[4 + m // 2]
                dpsA = psD.tile([64, 512], F32, tag="psDA", name="psDA")
                dpsB = psD.tile([64, 512], F32, tag="psDB", name="psDB")
                for dps, hrow in ((dpsA, 0), (dpsB, 64)):
                    nc.tensor.matmul(
                        dps[:],
                        xf_sb[hrow : hrow + 64, :],
                        yf8_sb[hrow : hrow + 64, :],
                        start=True,
                        stop=False,
                        tile_position=(hrow, 0),
                        skip_group_check=True,
                    )
                for nl in range(8):
                    ncol = (oct_ * 8 + nl) * 64
                    for dps, hrow in ((dpsA, 0), (dpsB, 64)):
                        nc.tensor.matmul(
                            dps[:, nl * 64 : (nl + 1) * 64],
                            kf[hrow : hrow + 64, ncol : ncol + 64],
                            qf[hrow : hrow + 64, ncol : ncol + 64],
                            start=False,
                            stop=nl == 7,
                            tile_position=(hrow, 0),
                            skip_group_check=True,
                        )
                for gg, dps in ((g, dpsA), (g + 1, dpsB)):
                    e_t = attL.tile([64, 512], BF16, tag="e", name="e")
                    nc.scalar.activation(
                        e_t[:], dps[:], mybir.ActivationFunctionType.Exp,
                        scale=EXPSCALE,
                    )
                    state[gg] = {"e": e_t, "m": m + (gg - g), "oct": oct_, "ch": ch}

            def emit_mid(g):
                # mm2 (+ones column -> sums row 64), reciprocal, on-chip
                # partition broadcast
                st = state[g]
                m, oct_, e_t, ch = st["m"], st["oct"], st["e"], st["ch"]
                _, v_sb, vlo_sb = chunk_tiles[ch]
                ops = psM.tile([65, 512], F32, tag="psM", name="psM")
                for nl in range(8):
                    tt = oct_ * 4 + nl // 2
                    if nl % 2 == 0:
                        lhsT = v_sb[tt][0:64, m * 65 : (m + 1) * 65]
                    else:
                        lhsT = vlo_sb[tt][:, m * 65 : (m + 1) * 65]
                    nc.tensor.matmul(
                        ops[:, nl * 64 : (nl + 1) * 64],
                        lhsT,
                        e_t[:, nl * 64 : (nl + 1) * 64],
                        start=True,
                        stop=True,
                    )
                r_t = attS.tile([1, 512], BF16, tag="s", name="s")
                with nc.allow_low_precision(reason="softmax recip in bf16"):
                    nc.vector.reciprocal(r_t[:], ops[64:65, :])
                norm = attL.tile([64, 512], BF16, tag="norm", name="norm")
                nc.gpsimd.partition_broadcast(norm[:], r_t[:])
                st["norm"] = norm
                st["ops"] = ops

            drain_alo = {}

            def emit_norm(g):
                # normalize (multiply by 1/sums) + at write
                st = state.pop(g)
                m, oct_, ch = st["m"], st["oct"], st["ch"]
                at_sb = get_at(ch)
                kt = m // 2
                if m % 2 == 0:
                    if K_DNE and ch == NCHUNK - 1 and (oct_ == 1 or K_DNE0):
                        # drain: DVE serializes the last norms while ACT and
                        # Pool idle - stage via ACT, multiply on Pool
                        oo = attL.tile([64, 512], BF16, tag="oo", name="oo")
                        nc.scalar.copy(oo[:], st["ops"][0:64, :])
                        nc.gpsimd.tensor_tensor(
                            at_sb[kt][0:64, oct_ * 512 : (oct_ + 1) * 512],
                            oo[:],
                            st["norm"][:],
                            mybir.AluOpType.mult,
                        )
                    else:
                        nc.vector.tensor_tensor(
                            at_sb[kt][0:64, oct_ * 512 : (oct_ + 1) * 512],
                            st["ops"][0:64, :],
                            st["norm"][:],
                            mybir.AluOpType.mult,
                        )
                else:
                    alo = attL.tile([64, 512], BF16, tag="alo", name="alo")
                    if K_DNO and ch == NCHUNK - 1 and oct_ == 1:
                        oo = attL.tile([64, 512], BF16, tag="oo", name="oo")
                        nc.scalar.copy(oo[:], st["ops"][0:64, :])
                        nc.gpsimd.tensor_tensor(
                            alo[:], oo[:], st["norm"][:], mybir.AluOpType.mult
                        )
                    else:
                        nc.vector.tensor_tensor(
                            alo[:], st["ops"][0:64, :], st["norm"][:],
                            mybir.AluOpType.mult,
                        )
                    if ch == NCHUNK - 1 and (m == 7 or (K_DR1 and m == 5 and oct_ == 1)
                            or (K_DR2 and oct_ == 1)):
                        # drain: oproj reads these tiles directly (via the
                        # wout2 duplicate) - the partition-shift DMAs would
                        # sit on the exit critical path
                        drain_alo[(m, oct_)] = alo
                        return
                    nc.sync.dma_start(
                        out=at_sb[kt][64:128, oct_ * 512 : (oct_ + 1) * 512],
                        in_=alo[:],
                    )

            oproj_part = {}

            def emit_oproj_p0(ch, tt, kts=range(3)):
                # pre-stage kt0..2 before the last at-tile is ready; callable
                # per-kt so the matmul bursts spread across iterations
                at_sb = chunk_at[ch]
                ps = oproj_part.get((ch, tt))
                if ps is None:
                    ps = psA.tile([128, 512], F32, tag="psA", name="psA")
                    oproj_part[(ch, tt)] = ps
                for kt in kts:
                    nc.tensor.matmul(
                        ps[:],
                        at_sb[kt][:, tt * 128 : (tt + 1) * 128],
                        wout_sb[:, kt, :],
                        start=(kt == 0),
                        stop=False,
                        skip_group_check=True,
                    )

            def drain_finish(tt, ps):
                ch = NCHUNK - 1
                t0 = ch * CHUNK + tt * 128
                o_t = attS.tile([128, C], BF16, tag="o", name="o")
                if tt < 6:
                    # all on ACT: DVE's recip/norm chain paces the drain
                    if tt % 2 == 0 or K_DCA:
                        nc.scalar.copy(o_t[:], ps[:])
                    else:
                        nc.vector.tensor_copy(o_t[:], ps[:])
                    eng = nc.sync if tt % 2 == 0 else nc.gpsimd
                    eng.dma_start(out=out_d.ap()[t0 : t0 + 128, :], in_=o_t[:])
                    return
                if tt == 7 and K_Q4:
                    # quarter the final tile: shortest possible exit chain,
                    # last DMA on a HWDGE queue (trailing SWDGE delays exit)
                    engs = ((nc.scalar, nc.sync), (nc.vector, nc.gpsimd),
                            (nc.scalar, nc.sync), (nc.vector, nc.scalar))
                    for q, (ce, de) in enumerate(engs):
                        c0 = q * 128
                        if ce is nc.vector:
                            nc.vector.tensor_copy(
                                o_t[:, c0 : c0 + 128], ps[:, c0 : c0 + 128]
                            )
                        else:
                            nc.scalar.copy(
                                o_t[:, c0 : c0 + 128], ps[:, c0 : c0 + 128]
                            )
                        de.dma_start(
                            out=out_d.ap()[t0 : t0 + 128, c0 : c0 + 128],
                            in_=o_t[:, c0 : c0 + 128],
                        )
                    return
                nc.scalar.copy(o_t[:, 0:256], ps[:, 0:256])
                nc.vector.tensor_copy(o_t[:, 256:512], ps[:, 256:512])
                nc.sync.dma_start(
                    out=out_d.ap()[t0 : t0 + 128, 0:256], in_=o_t[:, 0:256]
                )
                eng2 = nc.scalar if tt == 7 else nc.gpsimd
                eng2.dma_start(
                    out=out_d.ap()[t0 : t0 + 128, 256:512], in_=o_t[:, 256:512]
                )

            def emit_oproj_drain0(tt):
                # last chunk oct0: kt0-2 prestaged full (its alo DMAs land in
                # time); kt3 split so only the m7 alo tile is read directly
                ch = NCHUNK - 1
                at_sb = chunk_at[ch]
                ps = oproj_part.pop((ch, tt))
                nc.tensor.matmul(
                    ps[:],
                    at_sb[3][0:64, tt * 128 : (tt + 1) * 128],
                    wout_sb[0:64, 3, :],
                    start=False,
                    stop=False,
                    skip_group_check=True,
                )
                nc.tensor.matmul(
                    ps[:],
                    drain_alo[(7, 0)][:, (tt % 4) * 128 :][:, 0:128],
                    wout2_sb[:, 3, :],
                    start=False,
                    stop=True,
                    skip_group_check=True,
                )
                drain_finish(tt, ps)

            def emit_oproj_p01(tt):
                # oct1 prestage: full kt matmuls for those whose alo DMAs
                # land in time (kt0-1 under K_DR1, kt0-2 otherwise)
                ch = NCHUNK - 1
                at_sb = chunk_at[ch]
                ps = psA.tile([128, 512], F32, tag="psA", name="psA")
                for kt in range(2 if K_DR1 else 3):
                    nc.tensor.matmul(
                        ps[:],
                        at_sb[kt][:, tt * 128 : (tt + 1) * 128],
                        wout_sb[:, kt, :],
                        start=(kt == 0),
                        stop=False,
                        skip_group_check=True,
                    )
                oproj_part[(ch, tt)] = ps

            dr2_ps = {}

            def emit_dr2(kt):
                # full alo-direct oct1: stage kt's lo+hi K=64 halves across
                # all 4 tiles as their norm outputs arrive - zero DMA waits
                ch = NCHUNK - 1
                at_sb = chunk_at[ch]
                tts = [4, 5, 6, 7]
                if kt == 0:
                    for tt in tts:
                        dr2_ps[tt] = psA.tile(
                            [128, 512], F32, tag="psA", name="psA"
                        )
                for tt in tts:
                    nc.tensor.matmul(
                        dr2_ps[tt][:],
                        at_sb[kt][0:64, tt * 128 : (tt + 1) * 128],
                        wout_sb[0:64, kt, :],
                        start=(kt == 0),
                        stop=False,
                        skip_group_check=True,
                    )
                alo = drain_alo[(2 * kt + 1, 1)]
                for tt in tts:
                    c0 = (tt % 4) * 128
                    nc.tensor.matmul(
                        dr2_ps[tt][:],
                        alo[:, c0 : c0 + 128],
                        wout2_sb[:, kt, :],
                        start=False,
                        stop=(kt == 3),
                        skip_group_check=True,
                    )
                if kt == 3:
                    for tt in tts:
                        drain_finish(tt, dr2_ps.pop(tt))

            def emit_oproj_drain1():
                # oct1 finals: kt2/kt3 split, kt-major within tile pairs so
                # the in-order PE queue consumes operands in norm-arrival
                # order (m4, m5, m6, m7); nothing else needs the PE after
                ch = NCHUNK - 1
                at_sb = chunk_at[ch]
                tts = [4, 5, 6, 7]
                pss = {tt: oproj_part.pop((ch, tt)) for tt in tts}
                for kt in ((2, 3) if K_DR1 else (3,)):
                    for tt in tts:
                        nc.tensor.matmul(
                            pss[tt][:],
                            at_sb[kt][0:64, tt * 128 : (tt + 1) * 128],
                            wout_sb[0:64, kt, :],
                            start=False,
                            stop=False,
                            skip_group_check=True,
                        )
                    alo = drain_alo[(2 * kt + 1, 1)]
                    for tt in tts:
                        c0 = (tt % 4) * 128
                        nc.tensor.matmul(
                            pss[tt][:],
                            alo[:, c0 : c0 + 128],
                            wout2_sb[:, kt, :],
                            start=False,
                            stop=(kt == 3),
                            skip_group_check=True,
                        )
                for tt in tts:
                    drain_finish(tt, pss[tt])

            def emit_oproj(ch, tt):
                at_sb = chunk_at[ch]
                ps = oproj_part.pop((ch, tt), None)
                if ps is not None:
                    nc.tensor.matmul(
                        ps[:],
                        at_sb[3][:, tt * 128 : (tt + 1) * 128],
                        wout_sb[:, 3, :],
                        start=False,
                        stop=True,
                        skip_group_check=True,
                    )
                else:
                    ps = psA.tile([128, 512], F32, tag="psA", name="psA")
                    for kt in range(4):
                        nc.tensor.matmul(
                            ps[:],
                            at_sb[kt][:, tt * 128 : (tt + 1) * 128],
                            wout_sb[:, kt, :],
                            start=(kt == 0),
                            stop=(kt == 3),
                        )
                t0 = ch * CHUNK + tt * 128
                # PSUM -> SBUF split across ACT/DVE (gpsimd cannot read PSUM)
                o_t = attS.tile([128, C], BF16, tag="o", name="o")
                if ch == NCHUNK - 1 and tt >= 6:
                    # final tiles: halve the copy across ACT+DVE and the out
                    # DMA across SP+Pool so the exit chain shortens
                    nc.scalar.copy(o_t[:, 0:256], ps[:, 0:256])
                    nc.vector.tensor_copy(o_t[:, 256:512], ps[:, 256:512])
                    nc.sync.dma_start(
                        out=out_d.ap()[t0 : t0 + 128, 0:256], in_=o_t[:, 0:256]
                    )
                    eng2 = nc.scalar if tt == 7 else nc.gpsimd
                    eng2.dma_start(
                        out=out_d.ap()[t0 : t0 + 128, 256:512], in_=o_t[:, 256:512]
                    )
                    return
                if tt in OP_ACT:
                    nc.scalar.copy(o_t[:], ps[:])
                else:
                    nc.vector.tensor_copy(o_t[:], ps[:])
                eng = nc.sync if tt % 2 == 0 else nc.gpsimd
                eng.dma_start(out=out_d.ap()[t0 : t0 + 128, :], in_=o_t[:])

            # ---- stage-1 work scheduled just-in-time ----
            NG = NCHUNK * NUNITS
            PRE = 4
            sched = {}

            def at_iter(i, fn):
                sched.setdefault(i, []).append(fn)

            group_order = []
            for th in range(2):
                for ft in (0, 4, 1, 5, 2, 6, 3, 7):
                    group_order.append(ft * 2 + th)
                group_order.extend(16 + th * 4 + tt for tt in range(4))
            SPREADS = {
                # 2 groups/iter bursts (original)
                "a": [0, 0, 1, 1, 2, 2, 4, 4, 3, 3, 5, 5,
                      8, 8, 9, 9, 10, 10, 11, 11, 12, 12, 13, 13],
                # flat: 1 qk group/iter so the ACT copy stream never bursts
                "f": [0, 1, 2, 3, 4, 5, 6, 7,
                      8, 8, 9, 9,
                      9, 10, 11, 12, 13, 14, 15, 16,
                      16, 16, 17, 17],
                "g": [0, 1, 2, 3, 4, 5, 6, 7,
                      7, 8, 8, 9,
                      9, 10, 11, 12, 13, 14, 15, 16,
                      15, 16, 17, 17],
            }
            spread = SPREADS[os.environ.get("K_SPREAD", "a")]

            group_order0 = (
                [ft * 2 + 0 for ft in (0, 4, 1, 5, 2, 6, 3, 7)]
                + [16, 17, 18, 19]
                + [ft * 2 + 1 for ft in (0, 4, 1, 5, 2, 6, 3, 7)]
                + [20, 21, 22, 23]
            )
            spread0 = [-4, -4, -3, -3, -2, -2, -1, -1,
                       0, 0, 1, 1,
                       2, 2, 3, 3, 4, 4, 5, 5,
                       6, 6, 7, 7]

            def schedule_chunk(ch):
                emitters, tiles = stage1_groups(load_xt(ch), ch)
                chunk_tiles[ch] = tiles
                if ch == 0:
                    order, offs, base = group_order0, spread0, 0
                else:
                    order, offs, base = (
                        group_order, [o + K_SHIFT for o in spread],
                        (ch - 1) * NUNITS,
                    )
                for j, gi in enumerate(order):
                    at_iter(base + offs[j], emitters[gi])

            for i in range(-PRE, NG + 24):
                if i == -PRE:
                    schedule_chunk(0)
                if i == 2:
                    load_wout()
                for ch in range(1, NCHUNK):
                    if i == (ch - 1) * NUNITS:
                        schedule_chunk(ch)
                for fn in sched.get(i, []):
                    fn()
                if K_MIDF:
                    # mm2 first: unit i-1's mm2 jumps ahead of pair i's
                    # dots in the in-order PE queue
                    if 0 <= i - 1 < NG:
                        emit_mid(i - 1)
                    if 0 <= i < NG and i % 2 == 0:
                        emit_front_pair(i)
                else:
                    if 0 <= i < NG and i % 2 == 0:
                        emit_front_pair(i)
                    if 0 <= i - 1 < NG:
                        emit_mid(i - 1)
                gg = i - K_LAG
                if 0 <= gg < NG:
                    if gg >= NG - K_OF:
                        # drain: odd head first so the partition-shift DMA
                        # (alo) starts earlier; both norms of the pair at once
                        if gg % 2 == 0:
                            emit_norm(gg + 1)
                            emit_norm(gg)
                    else:
                        emit_norm(gg)
                for ch in range(NCHUNK - 1):
                    cb = ch * NUNITS
                    if K_P2 and ch in (1, 2):
                        # prestage kt0-2 of the o1 wave so only the cheap
                        # kt3 finals land in the next chunk's dots window
                        if i == cb + K_O1 - 2:
                            emit_oproj_p0(ch, 4)
                            emit_oproj_p0(ch, 5)
                        if i == cb + K_O1 - 1:
                            emit_oproj_p0(ch, 6)
                            emit_oproj_p0(ch, 7)
                    if cb + K_O0 <= i <= cb + K_O0 + 3:
                        emit_oproj(ch, i - cb - K_O0)
                    if cb + K_O1 <= i <= cb + K_O1 + 3:
                        emit_oproj(ch, i - cb - K_O1 + 4)
                cb3 = (NCHUNK - 1) * NUNITS
                if cb3 + K_O0 <= i <= cb3 + K_O0 + 3:
                    emit_oproj_drain0(i - cb3 - K_O0)
                if K_P0S:
                    for kt in range(3):
                        if i == cb3 + K_P0A + kt - 2:
                            emit_oproj_p0(NCHUNK - 1, 0, [kt])
                            emit_oproj_p0(NCHUNK - 1, 1, [kt])
                        if i == cb3 + K_P0B + kt - 2:
                            emit_oproj_p0(NCHUNK - 1, 2, [kt])
                            emit_oproj_p0(NCHUNK - 1, 3, [kt])
                else:
                    if i == cb3 + K_P0A:
                        emit_oproj_p0(NCHUNK - 1, 0)
                        emit_oproj_p0(NCHUNK - 1, 1)
                    if i == cb3 + K_P0B:
                        emit_oproj_p0(NCHUNK - 1, 2)
                        emit_oproj_p0(NCHUNK - 1, 3)
                if K_DR2:
                    if cb3 + K_LO1 - 3 <= i <= cb3 + K_LO1:
                        emit_dr2(i - cb3 - K_LO1 + 3)
                else:
                    if i == cb3 + K_P1A:
                        emit_oproj_p01(4)
                        emit_oproj_p01(5)
                    if i == cb3 + K_P1B:
                        emit_oproj_p01(6)
                        emit_oproj_p01(7)
                    if i == cb3 + K_LO1:
                        emit_oproj_drain1()

    nc.finalize()
    return nc


def _get_nc():
    global _NC_CACHE
    if _NC_CACHE is None:
        _NC_CACHE = build_nc()
    return _NC_CACHE


def _bias_factors(pos_emb: np.ndarray):
    """B8*2^20 = X^T Y (factors x2^10 each), duplicated across row halves."""
    idx = np.array([[i, j] for i in range(WS) for j in range(WS)])
    rel = idx[None, :, :] - idx[:, None, :] + WS - 1
    bias = pos_emb[rel[:, :, 0], rel[:, :, 1]]            # [q, k]
    b8 = bias.T.astype(np.float64) * 8.0                  # [k, q] * 1/SCALE
    u, s, vt = np.linalg.svd(b8)
    x = (u * np.sqrt(s)).T * SPROD                        # [r=64, k=64]
    y = (np.sqrt(s)[:, None] * vt) * SPROD                # [r=64, q=64]
    xf = np.tile(x, (2, 1)).astype(ml_dtypes.bfloat16)    # [128, 64]
    yf8 = np.tile(np.tile(y, (1, 8)), (2, 1)).astype(ml_dtypes.bfloat16)
    return np.ascontiguousarray(xf), np.ascontiguousarray(yf8)


def _pack8(a, rows):
    """[512, n] -> fp8 [128, 4*n] with c = 256*kt2 + 128*i + p."""
    n = a.shape[1]
    a = a.reshape(2, 2, 128, n).transpose(2, 0, 1, 3).reshape(128, 4 * n)
    return np.ascontiguousarray(a.astype(NPF8))


def host_prep(x, w_qkv, pos_emb, w_out, b_out):
    """Shard + quantize + lay out the inputs: one in_map per core."""
    x = np.asarray(x, dtype=np.float32)
    w_qkv = np.asarray(w_qkv, dtype=np.float32)
    pos_emb = np.asarray(pos_emb, dtype=np.float32)
    w_out = np.ascontiguousarray(np.asarray(w_out, dtype=np.float32))

    nh = H // WS
    # [slice, c, tok'] with tok' in window order (nh, nw, wsh, wsw)
    xt = x.reshape(B * L, nh, WS, nh, WS, C).transpose(0, 5, 1, 3, 2, 4)
    xt = np.ascontiguousarray(xt.reshape(B * L, C, TOK)) * SX
    xh8 = xt.astype(NPF8)
    xl8 = (xt - xh8.astype(np.float32)).astype(NPF8)

    wqk = w_qkv[:, : 2 * INNER] * SW
    wv = w_qkv[:, 2 * INNER :] * SW
    wv8h = wv.astype(NPF8)
    wv8l = (wv - wv8h.astype(np.float32)).astype(NPF8)

    wqk8 = _pack8(wqk, 512)
    wv8h_p = _pack8(wv8h.astype(np.float32), 512)
    wv8l_p = _pack8(wv8l.astype(np.float32), 512)
    wout = np.ascontiguousarray(w_out.astype(ml_dtypes.bfloat16))
    wout2 = w_out.reshape(4, 128, C)[:, 64:128].transpose(1, 0, 2)
    wout2 = np.ascontiguousarray(
        wout2.reshape(64, 4 * C).astype(ml_dtypes.bfloat16)
    )
    xf, yf8 = _bias_factors(pos_emb)

    maps = []
    for s in range(NCORES):
        maps.append({
            "xt8h": _pack8(xh8[s].astype(np.float32), TOK),
            "xt8l": _pack8(xl8[s].astype(np.float32), TOK),
            "wqk8": wqk8,
            "wv8h": wv8h_p,
            "wv8l": wv8l_p,
            "wout": wout,
            "wout2": wout2,
            "xf": xf,
            "yf8": yf8,
        })
    return maps


def host_post(out_slices, b_out):
    """[NCORES x (tok', c)] window-ordered -> [b, l, h, w, c] (+ b_out)."""
    nh = H // WS
    out = np.stack([np.asarray(o) for o in out_slices]).astype(np.float32)
    out = out.reshape(B * L, nh, nh, WS, WS, C).transpose(0, 1, 3, 2, 4, 5)
    out = out.reshape(B, L, H, W, C)
    return np.ascontiguousarray(out + np.asarray(b_out, dtype=np.float32))


def kernel(x, w_qkv, pos_emb, w_out, b_out):
    in_maps = host_prep(x, w_qkv, pos_emb, w_out, b_out)
    nc = _get_nc()
    res = run_bass_kernel_spmd(nc, in_maps, list(range(NCORES)))
    return host_post([res.results[s]["out"] for s in range(NCORES)], b_out)
